# revision 1
# baseline (speedup 1.0000x reference)
"""Distributed Trainium2 Bass kernel for nn_AdaptiveGNN (gnn_message_passing).

Strategy (8 NeuronCores, node-sharded graph parallel):
  - Nodes sharded row-wise: 6250/core, padded to 6272 = 49*128.
  - Per-node payload row kept in a bf16 DRAM table H [50176, 256] (512B rows):
    [h(128) | onehot(center)(8) | logits(8) | zero pad(112)]. Each iteration:
    AllGather the local shard, then gather the 16 neighbor rows per node with
    dma_gather (custom SWDGE bulk gather) and reduce over the 16 with DVE.
  - int16 gather indices cap the addressable range at 32768 rows, so each
    chunk runs TWO windowed passes (rows [0,32768) and [17408,50176)); edges
    outside a pass's window point at a known all-zero row (a pad row), so the
    k-reduce of (passA + passB) is exact with no masking.
  - softmax is argmax-invariant and pred = center[neighbors], so the class
    histogram cnts IS the aggregated one-hot, and f1 = cnts[i, center_i].
  - logits are maintained incrementally: logits += gate * agg(h@Wy), avoiding
    matmuls/transposes inside the loop (iteration 0 computes feat@W, feat@(W@Wy)).
  - LayerNorm(f1/f2) stats are global: AllGather of 4 partial sums per iteration.
  - Final classifier: Wc sharded along N*D; DVE multiply + ones-matmul PSUM
    accumulation; AllReduce of the [1,8] output.
"""

from contextlib import ExitStack

import numpy as np

import concourse.bass as bass
import concourse.mybir as mybir
from concourse import bacc, library_config
from concourse.bass import AP
from concourse.bass_utils import run_bass_kernel_spmd

F32 = mybir.dt.float32
BF16 = mybir.dt.bfloat16
I16 = mybir.dt.int16

N = 50000
K = 16
DH = 128
C = 8
NCORES = 8
SH = 6250            # nodes per core
P = 128
M = 49               # free-dim groups per core
SHP = P * M          # 6272 padded nodes per core
E = DH + 2 * C       # 144: h | onehot | logits (f32 master cols)
EP = DH + 3 * C      # 152: h | onehot | logits_hi | logits_lo (table payload)
EB = 256             # bf16 table row (512B): payload + zero pad
GN = NCORES * SHP    # 50176 table rows
WB = 17408           # window B base row (both windows 32768 rows)
DUM_A = SH           # rank0 pad row (all zero) inside window A
DUM_B = 3 * SHP + SH - WB  # rank3 pad row, window-B-relative
EPS = 1e-5
CLAMP = 1e-5
import os
NITER = int(os.environ.get('KERNEL_NITER', '3'))
SKIP_CLS = bool(int(os.environ.get('KERNEL_SKIP_CLS', '0')))
CH_M = 2             # m-groups per gather chunk
NCH = (M + CH_M - 1) // CH_M  # 25 chunks (24x2 + 1x1)
AX = mybir.AxisListType
OP = mybir.AluOpType
AF = mybir.ActivationFunctionType


class Sem:
    def __init__(self, handle, step):
        self.h = handle
        self.step = step
        self.n = 0

    def inc(self, instr):
        instr.then_inc(self.h, self.step)
        self.n += self.step
        return self.n


def build_program():
    nc = bacc.Bacc(num_devices=NCORES)

    # ---------------- DRAM parameters ----------------
    feat_s = nc.declare_dram_parameter("feat_s", [SHP, DH], F32, isOutput=False)
    nbidxA = nc.declare_dram_parameter("nbidxA", [P, SHP], I16, isOutput=False)
    nbidxB = nc.declare_dram_parameter("nbidxB", [P, SHP], I16, isOutput=False)
    w_gcn = nc.declare_dram_parameter("w_gcn", [DH, DH], F32, isOutput=False)
    b_gcn = nc.declare_dram_parameter("b_gcn", [DH], F32, isOutput=False)
    wy = nc.declare_dram_parameter("wy", [DH, C], F32, isOutput=False)
    by = nc.declare_dram_parameter("by", [C], F32, isOutput=False)
    taus = nc.declare_dram_parameter("taus", [2], F32, isOutput=False)
    wc_s = nc.declare_dram_parameter("wc_s", [SH * DH, C], F32, isOutput=False)
    bc8 = nc.declare_dram_parameter("bc8", [C], F32, isOutput=False)
    out_ext = nc.declare_dram_parameter("out", [1, C], F32, isOutput=True)

    # ---------------- internal DRAM ----------------
    ag_in = nc.dram_tensor("ag_in", [SHP, EP], BF16)
    h_sh = nc.dram_tensor("h_sh", [GN, EP], BF16, addr_space="Shared")
    h_full = nc.dram_tensor("h_full", [GN, EB], BF16)
    stats_in = nc.dram_tensor("stats_in", [4], F32)
    stats_out = nc.dram_tensor("stats_out", [4 * NCORES], F32, addr_space="Shared")
    cls_in = nc.dram_tensor("cls_in", [C], F32)
    cls_out = nc.dram_tensor("cls_out", [C], F32, addr_space="Shared")

    # ---------------- SBUF ----------------
    sb_h = nc.alloc_sbuf_tensor("sb_h", [P, M, E], F32)       # f32 master
    sb_hb = nc.alloc_sbuf_tensor("sb_hb", [P, M, EP], BF16)   # bf16 staging (packed)
    acc = nc.alloc_sbuf_tensor("acc", [P, M, EP], F32)        # reduced aggregation
    sb_idxA = nc.alloc_sbuf_tensor("sb_idxA", [P, SHP], I16)
    sb_idxB = nc.alloc_sbuf_tensor("sb_idxB", [P, SHP], I16)
    # sb_big (feat staging / h-update scratch) aliases the bufA gather buffers:
    # feat is fully consumed before the first gather, and the update scratch is
    # only live after the last reduce of an iteration and before the next
    # iteration's first gather (which waits on the h-update via dv).
    sb_un = nc.alloc_sbuf_tensor("sb_un", [P, 2 * CH_M * K * EB // 2], F32)
    un_off = nc.lookup_mls(sb_un).memorylocations[0].addr
    bufA = [
        nc.alloc_sbuf_tensor_at(f"bufA{i}", [P, CH_M * K, EB], BF16,
                                offset=un_off + i * CH_M * K * EB * 2)
        for i in range(2)
    ]
    sb_big = nc.alloc_sbuf_tensor_at("sb_big", [P, M, DH], F32, offset=un_off)
    bufB = [nc.alloc_sbuf_tensor(f"bufB{i}", [P, CH_M * K, EB], BF16) for i in range(2)]
    sb_rB = nc.alloc_sbuf_tensor("sb_rB", [P, CH_M, EP], F32)
    sb_wc = [nc.alloc_sbuf_tensor(f"sb_wc{i}", [P, DH * C], F32) for i in range(2)]
    sb_tmp = [nc.alloc_sbuf_tensor(f"sb_tmp{i}", [P, DH * C], F32) for i in range(2)]

    sb_W = nc.alloc_sbuf_tensor("sb_W", [DH, DH], F32)
    sb_WT = nc.alloc_sbuf_tensor("sb_WT", [DH, DH], F32)
    sb_Wy = nc.alloc_sbuf_tensor("sb_Wy", [DH, C], F32)
    sb_WWy = nc.alloc_sbuf_tensor("sb_WWy", [DH, C], F32)
    sb_ft = nc.alloc_sbuf_tensor("sb_ft", [P, 4 * P], F32)
    ident = nc.alloc_sbuf_tensor("ident", [P, P], F32)

    sb_bgrow = nc.alloc_sbuf_tensor("sb_bgrow", [1, DH], F32)
    sb_bgcol = nc.alloc_sbuf_tensor("sb_bgcol", [DH, 1], F32)
    sb_byrow = nc.alloc_sbuf_tensor("sb_byrow", [1, C], F32)
    sb_taurow = nc.alloc_sbuf_tensor("sb_taurow", [1, 2], F32)
    sb_bc8row = nc.alloc_sbuf_tensor("sb_bc8row", [1, C], F32)
    sb_c8row = nc.alloc_sbuf_tensor("sb_c8row", [1, C], F32)
    sb_pack = nc.alloc_sbuf_tensor("sb_pack", [1, DH + C + 2], F32)
    sb_bcast = nc.alloc_sbuf_tensor("sb_bcast", [P, DH + C + 2], F32)

    ones_col = nc.alloc_sbuf_tensor("ones_col", [P, 1], F32)
    ones_row = nc.alloc_sbuf_tensor("ones_row", [1, P], F32)
    padmask = nc.alloc_sbuf_tensor("padmask", [P, M], F32)

    sb_mx = nc.alloc_sbuf_tensor("sb_mx", [P, M], F32)
    sb_f1 = nc.alloc_sbuf_tensor("sb_f1", [P, M], F32)
    sb_f2 = nc.alloc_sbuf_tensor("sb_f2", [P, M], F32)
    sb_z1 = nc.alloc_sbuf_tensor("sb_z1", [P, M], F32)
    sb_z2 = nc.alloc_sbuf_tensor("sb_z2", [P, M], F32)
    sb_z = nc.alloc_sbuf_tensor("sb_z", [P, M], F32)
    sb_oldz = nc.alloc_sbuf_tensor("sb_oldz", [P, M], F32)
    sb_gate = nc.alloc_sbuf_tensor("sb_gate", [P, M], F32)
    sb_junk = nc.alloc_sbuf_tensor("sb_junk", [P, M], F32)
    sb_t8 = nc.alloc_sbuf_tensor("sb_t8", [P, M, C], F32)
    sb_q = nc.alloc_sbuf_tensor("sb_q", [P, M, C], F32)
    sb_lnq = nc.alloc_sbuf_tensor("sb_lnq", [P, M, C], F32)
    sb_p4 = nc.alloc_sbuf_tensor("sb_p4", [P, 4], F32)
    sb_s4 = nc.alloc_sbuf_tensor("sb_s4", [1, 4], F32)
    sb_s32 = nc.alloc_sbuf_tensor("sb_s32", [1, 4 * NCORES], F32)
    sb_sum4 = nc.alloc_sbuf_tensor("sb_sum4", [1, 4], F32)
    sb_mean = nc.alloc_sbuf_tensor("sb_mean", [1, 2], F32)
    sb_msq = nc.alloc_sbuf_tensor("sb_msq", [1, 2], F32)
    sb_var = nc.alloc_sbuf_tensor("sb_var", [1, 2], F32)
    sb_sd = nc.alloc_sbuf_tensor("sb_sd", [1, 2], F32)
    sb_rstd = nc.alloc_sbuf_tensor("sb_rstd", [1, 2], F32)
    sb_nrstd = nc.alloc_sbuf_tensor("sb_nrstd", [1, 2], F32)
    sb_mr = nc.alloc_sbuf_tensor("sb_mr", [1, 2], F32)
    sb_bias = nc.alloc_sbuf_tensor("sb_bias", [1, 2], F32)
    sb_pack4 = nc.alloc_sbuf_tensor("sb_pack4", [1, 4], F32)
    sb_sc = nc.alloc_sbuf_tensor("sb_sc", [P, 4], F32)
    sb_r0 = nc.alloc_sbuf_tensor("sb_r0", [1, C], F32)
    sb_r1 = nc.alloc_sbuf_tensor("sb_r1", [1, C], F32)
    sb_o = nc.alloc_sbuf_tensor("sb_o", [1, C], F32)

    # ---------------- PSUM ----------------
    ps_t4 = nc.alloc_psum_tensor("ps_t4", [P, 4 * P], F32)
    ps_g4 = nc.alloc_psum_tensor("ps_g4", [P, 4 * P], F32)
    ps_w4 = nc.alloc_psum_tensor("ps_w4", [P, 4 * C], F32)
    ps_s = nc.alloc_psum_tensor("ps_s", [1, C], F32)
    ps_mi = nc.alloc_psum_tensor("ps_mi", [P, DH + C + 2], F32)
    ps_c0 = nc.alloc_psum_tensor("ps_c0", [1, 512], F32)
    ps_c1 = nc.alloc_psum_tensor("ps_c1", [1, 512], F32)

    hS = M * E

    def hv(c0, n):
        return AP(sb_h, c0, [[hS, P], [E, M], [1, n]])

    aS = M * EP

    def accv(c0, n):
        return AP(acc, c0, [[aS, P], [EP, M], [1, n]])

    def gatev(n):
        return AP(sb_gate, 0, [[M, P], [1, M], [0, n]])

    rg = [list(range(NCORES))]

    with ExitStack() as st:
        block = st.enter_context(nc.Block())
        dS = Sem(st.enter_context(nc.semaphore("dS")), 16)
        cc = Sem(st.enter_context(nc.semaphore("cc")), 1)
        pe = Sem(st.enter_context(nc.semaphore("pe")), 1)
        dv = Sem(st.enter_context(nc.semaphore("dv")), 1)
        ac = Sem(st.enter_context(nc.semaphore("ac")), 1)
        gp = Sem(st.enter_context(nc.semaphore("gp")), 1)
        gg = [Sem(st.enter_context(nc.semaphore(f"gg{i}")), 16) for i in range(2)]
        dW = [Sem(st.enter_context(nc.semaphore(f"dW{i}")), 16) for i in range(2)]
        dZ = Sem(st.enter_context(nc.semaphore("dZ")), 16)

        # ================= INIT =================
        def _(s):
            e = nc.sync
            loads = [
                e.dma_start(out=sb_W[:, :], in_=w_gcn[:, :]),
                e.dma_start(out=sb_Wy[:, :], in_=wy[:, :]),
                e.dma_start(out=sb_bgrow[:, :], in_=AP(b_gcn, 0, [[DH, 1], [1, DH]])),
                e.dma_start(out=sb_bgcol[:, :], in_=AP(b_gcn, 0, [[1, DH], [1, 1]])),
                e.dma_start(out=sb_byrow[:, :], in_=AP(by, 0, [[C, 1], [1, C]])),
                e.dma_start(out=sb_taurow[:, :], in_=AP(taus, 0, [[2, 1], [1, 2]])),
                e.dma_start(out=sb_bc8row[:, :], in_=AP(bc8, 0, [[C, 1], [1, C]])),
                e.dma_start(out=sb_idxA[:, :], in_=nbidxA[:, :]),
                e.dma_start(out=sb_idxB[:, :], in_=nbidxB[:, :]),
                e.dma_start(
                    out=sb_big[:, :, :],
                    in_=AP(feat_s, 0, [[DH, P], [P * DH, M], [1, DH]]),
                ),
            ]
            for i in loads:
                dS.inc(i)
        block.sync(_)
        thr_init = dS.n

        def _(v):
            e = nc.vector
            e.memset(ones_col[:, :], 1.0)
            e.memset(ones_row[:, :], 1.0)
            e.memset(sb_oldz[:, :], 1.0)
            dv.inc(e.memset(sb_hb[:, :, :], 0.0))
        block.vector(_)
        thr_hb0 = dv.n

        # one-time zero of the gather table (pad cols 152:256 stay zero forever)
        def _(s):
            e = nc.sync
            e.wait_ge(dv.h, thr_hb0)
            # zero pad cols 152:256 of the whole table (cols 0:152 are fully
            # rewritten by the per-iteration restride)
            rpp = GN // P  # 392 rows per partition-stripe
            pc = EB - EP   # 104 pad cols
            nr = 56        # rows per DMA so the zero-source fits in sb_hb
            for j in range(rpp // nr):
                dZ.inc(e.dma_start(
                    out=AP(h_full, EP + j * nr * EB, [[rpp * EB, P], [EB, nr], [1, pc]]),
                    in_=AP(sb_hb, 0, [[M * EP, P], [pc, nr], [1, pc]]),
                ))
        block.sync(_)
        thr_agz = dZ.n

        # identity + padmask on gpsimd (affine_select is gpsimd-only)
        def _(g):
            e = nc.gpsimd
            e.load_library(library_config.mlp)
            e.memset(ident[:, :], 0.0).then_inc(gp.h, 1)
            e.memset(padmask[:, :], 1.0).then_inc(gp.h, 1)
            gp.n += 2
            e.wait_ge(gp.h, gp.n)
            gp.inc(e.affine_select(
                out=ident[:, :], in_=ident[:, :],
                compare_op=OP.not_equal, fill=1.0, base=0,
                pattern=[[-1, P]], channel_multiplier=1,
            ))
            e.wait_ge(gp.h, gp.n)
            gp.inc(e.affine_select(
                out=padmask[:, :], in_=padmask[:, :],
                compare_op=OP.is_ge, fill=0.0, base=SH - 1,
                pattern=[[-P, M]], channel_multiplier=-1,
            ))
        block.gpsimd(_)
        thr_ident = gp.n

        # W^T, WWy = W @ Wy ; bgWy = b_gcn @ Wy ; broadcast pack
        def _(t):
            e = nc.tensor
            e.wait_ge(dS.h, thr_init)
            e.wait_ge(gp.h, thr_ident)
            pe.inc(e.transpose(out=ps_t4[:, 0:P], in_=sb_W[:, :], identity=ident[:, :]))
        block.tensor(_)
        thr_wt = pe.n

        def _(v):
            e = nc.vector
            e.wait_ge(pe.h, thr_wt)
            e.drain()
            dv.inc(e.tensor_copy(out=sb_WT[:, :], in_=ps_t4[:, 0:P]))
        block.vector(_)
        thr_wtc = dv.n

        def _(t):
            e = nc.tensor
            e.wait_ge(dv.h, thr_wtc)
            e.matmul(out=ps_w4[:, 0:C], lhsT=sb_WT[:, :], rhs=sb_Wy[:, :], start=True, stop=True)
            pe.inc(e.matmul(out=ps_s[0:1, 0:C], lhsT=sb_bgcol[:, :], rhs=sb_Wy[:, :], start=True, stop=True))
        block.tensor(_)
        thr_wwy = pe.n

        def _(v):
            e = nc.vector
            e.wait_ge(pe.h, thr_wwy)
            e.drain()
            e.tensor_copy(out=sb_WWy[:, :], in_=ps_w4[:, 0:C])
            e.tensor_copy(out=sb_c8row[:, :], in_=ps_s[0:1, 0:C])
            e.drain()
            e.tensor_add(out=sb_c8row[:, :], in0=sb_c8row[:, :], in1=sb_byrow[:, :])
            e.drain()
            e.tensor_copy(out=sb_pack[0:1, 0:DH], in_=sb_bgrow[:, :])
            e.tensor_copy(out=sb_pack[0:1, DH:DH + C], in_=sb_c8row[:, :])
            dv.inc(e.tensor_copy(out=sb_pack[0:1, DH + C:DH + C + 2], in_=sb_taurow[:, :]))
        block.vector(_)
        thr_pk = dv.n

        def _(t):
            e = nc.tensor
            e.wait_ge(dv.h, thr_pk)
            pe.inc(e.matmul(out=ps_mi[:, :], lhsT=ones_row[:, :], rhs=sb_pack[:, :], start=True, stop=True))
        block.tensor(_)
        thr_bc0 = pe.n

        def _(v):
            e = nc.vector
            e.wait_ge(pe.h, thr_bc0)
            e.drain()
            dv.inc(e.tensor_copy(out=sb_bcast[:, :], in_=ps_mi[:, :]))
        block.vector(_)
        thr_bcast = dv.n
        bg_rep = AP(sb_bcast, 0, [[DH + C + 2, P], [0, M], [1, DH]])
        c8_rep = AP(sb_bcast, DH, [[DH + C + 2, P], [0, M], [1, C]])

        # ============ ITER 0: stage bf16 [feat@W | 0 | feat@WWy] ============
        batches = []
        g0 = 0
        while g0 < M:
            batches.append((g0, min(4, M - g0)))
            g0 += 4

        thr_tr = {}
        thr_ftc = {}
        thr_mm = {}
        thr_gc = {}
        for bi, (g0, nbg) in enumerate(batches):
            def _(t, bi=bi, g0=g0, nbg=nbg):
                e = nc.tensor
                if bi == 0:
                    e.wait_ge(dv.h, thr_bcast)
                else:
                    e.wait_ge(dv.h, thr_gc[bi - 1])
                last = None
                for j in range(nbg):
                    if j:
                        e.drain()
                    last = e.transpose(
                        out=ps_t4[:, j * P:(j + 1) * P],
                        in_=sb_big[:, g0 + j, :],
                        identity=ident[:, :],
                    )
                thr_tr[bi] = pe.inc(last)
            block.tensor(_)

            def _(v, bi=bi, nbg=nbg):
                e = nc.vector
                e.wait_ge(pe.h, thr_tr[bi])
                e.drain()
                thr_ftc[bi] = dv.inc(
                    e.tensor_copy(out=sb_ft[:, 0:nbg * P], in_=ps_t4[:, 0:nbg * P])
                )
            block.vector(_)

            def _(t, bi=bi, nbg=nbg):
                e = nc.tensor
                e.wait_ge(dv.h, thr_ftc[bi])
                last = None
                for j in range(nbg):
                    if j:
                        e.drain()
                    e.matmul(
                        out=ps_g4[:, j * P:(j + 1) * P],
                        lhsT=sb_ft[:, j * P:(j + 1) * P],
                        rhs=sb_W[:, :], start=True, stop=True,
                    )
                    last = e.matmul(
                        out=ps_w4[:, j * C:(j + 1) * C],
                        lhsT=sb_ft[:, j * P:(j + 1) * P],
                        rhs=sb_WWy[:, :], start=True, stop=True,
                    )
                thr_mm[bi] = pe.inc(last)
            block.tensor(_)

            def _(v, bi=bi, g0=g0, nbg=nbg):
                e = nc.vector
                e.wait_ge(pe.h, thr_mm[bi])
                if bi == 0:
                    e.wait_ge(dZ.h, thr_agz)  # ag_in zero DMA done reading sb_hb
                e.drain()
                e.tensor_copy(
                    out=AP(sb_hb, g0 * EP, [[M * EP, P], [EP, nbg], [1, DH]]),
                    in_=AP(ps_g4, 0, [[4 * P, P], [P, nbg], [1, P]]),
                )
                e.tensor_copy(
                    out=AP(sb_hb, g0 * EP + DH + C, [[M * EP, P], [EP, nbg], [1, C]]),
                    in_=AP(ps_w4, 0, [[4 * C, P], [C, nbg], [1, C]]),
                )
                e.drain()
                # logits_lo = gWy - bf16(gWy)
                thr_gc[bi] = dv.inc(e.tensor_tensor(
                    out=AP(sb_hb, g0 * EP + DH + 2 * C, [[M * EP, P], [EP, nbg], [1, C]]),
                    in0=AP(ps_w4, 0, [[4 * C, P], [C, nbg], [1, C]]),
                    in1=AP(sb_hb, g0 * EP + DH + C, [[M * EP, P], [EP, nbg], [1, C]]),
                    op=OP.subtract,
                ))
            block.vector(_)
        thr_iter0 = dv.n

        # stage iter0 payload cols 0:144 to ag_in
        def _(s):
            e = nc.sync
            e.wait_ge(dv.h, thr_iter0)
            e.wait_ge(dZ.h, thr_agz)
            dS.inc(e.dma_start(
                out=AP(ag_in, 0, [[EP, P], [P * EP, M], [1, EP]]),
                in_=sb_hb[:, :, :],
            ))
        block.sync(_)
        thr_ag0in = dS.n

        # ---------------- AG + chunked gather/reduce ----------------
        def emit_ag_gather(wait_dS_thr, first_dv_thr):
            thr_g = {}
            thr_red = {}

            HH = SHP // 2   # 3136 rows per half-shard
            def _(g):
                e = nc.gpsimd
                e.wait_ge(dS.h, wait_dS_thr)
                cc.inc(e.collective_compute(
                    "AllGather", OP.bypass, replica_groups=rg,
                    ins=[AP(ag_in, 0, [[EP, HH], [1, EP]])],
                    outs=[AP(h_sh, 0, [[EP, NCORES * HH], [1, EP]])],
                ))
                cc.inc(e.collective_compute(
                    "AllGather", OP.bypass, replica_groups=rg,
                    ins=[AP(ag_in, HH * EP, [[EP, HH], [1, EP]])],
                    outs=[AP(h_sh, NCORES * HH * EP, [[EP, NCORES * HH], [1, EP]])],
                ))
            block.gpsimd(_)
            c1, c2 = cc.n - 1, cc.n

            def _(s):
                e = nc.sync
                e.wait_ge(cc.h, c1)
                if wait_dS_thr == thr_ag0in:
                    e.wait_ge(dZ.h, thr_agz)  # table zero-init done
                # AG1 output block r = rank r's rows 0:HH -> table rows r*SHP+0:HH
                dS.inc(e.dma_start(
                    out=AP(h_full, 0, [[SHP * EB, NCORES], [EB, HH], [1, EP]]),
                    in_=AP(h_sh, 0, [[HH * EP, NCORES], [EP, HH], [1, EP]]),
                ))
                e.wait_ge(cc.h, c2)
                dS.inc(e.dma_start(
                    out=AP(h_full, HH * EB, [[SHP * EB, NCORES], [EB, HH], [1, EP]]),
                    in_=AP(h_sh, NCORES * HH * EP, [[HH * EP, NCORES], [EP, HH], [1, EP]]),
                ))
            block.sync(_)
            thr_rs = dS.n

            def _(g):
                e = nc.gpsimd
                e.wait_ge(dS.h, thr_rs)
            block.gpsimd(_)

            def emit_gather(c):
                def _(g, c=c):
                    e = nc.gpsimd
                    nm = CH_M if c < NCH - 1 else M - CH_M * (NCH - 1)
                    ni = nm * K * P
                    if c >= 2:
                        e.wait_ge(dv.h, thr_red[c - 2])
                    elif first_dv_thr is not None:
                        e.wait_ge(dv.h, first_dv_thr)
                    col0 = (c * CH_M * K * P) // 16
                    ncol = ni // 16
                    e.dma_gather(
                        out_ap=bufA[c % 2][:, 0:nm * K, :],
                        in_ap=AP(h_full, 0, [[EB, 32768], [1, EB]]),
                        idxs_ap=sb_idxA[:, col0:col0 + ncol],
                        num_idxs=ni, num_idxs_reg=ni, elem_size=EB,
                        single_packet=False,
                    ).then_inc(gg[c % 2].h, 16)
                    e.dma_gather(
                        out_ap=bufB[c % 2][:, 0:nm * K, :],
                        in_ap=AP(h_full, WB * EB, [[EB, 32768], [1, EB]]),
                        idxs_ap=sb_idxB[:, col0:col0 + ncol],
                        num_idxs=ni, num_idxs_reg=ni, elem_size=EB,
                        single_packet=False,
                    ).then_inc(gg[c % 2].h, 16)
                    gg[c % 2].n += 32
                    thr_g[c] = gg[c % 2].n
                block.gpsimd(_)

            def emit_reduce(c):
                def _(v, c=c):
                    e = nc.vector
                    nm = CH_M if c < NCH - 1 else M - CH_M * (NCH - 1)
                    e.wait_ge(gg[c % 2].h, thr_g[c])
                    e.drain()
                    e.tensor_reduce(
                        out=AP(acc, c * CH_M * EP, [[aS, P], [EP, nm], [1, EP]]),
                        in_=AP(bufA[c % 2], 0, [[CH_M * K * EB, P], [K * EB, nm], [1, EP], [EB, K]]),
                        axis=AX.X, op=OP.add,
                    )
                    e.tensor_reduce(
                        out=AP(sb_rB, 0, [[CH_M * EP, P], [EP, nm], [1, EP]]),
                        in_=AP(bufB[c % 2], 0, [[CH_M * K * EB, P], [K * EB, nm], [1, EP], [EB, K]]),
                        axis=AX.X, op=OP.add,
                    )
                    e.drain()
                    thr_red[c] = dv.inc(e.tensor_tensor(
                        out=AP(acc, c * CH_M * EP, [[aS, P], [EP, nm], [1, EP]]),
                        in0=AP(acc, c * CH_M * EP, [[aS, P], [EP, nm], [1, EP]]),
                        in1=AP(sb_rB, 0, [[CH_M * EP, P], [EP, nm], [1, EP]]),
                        op=OP.add,
                    ))
                block.vector(_)

            emit_gather(0)
            emit_gather(1)
            for c in range(2, NCH):
                emit_reduce(c - 2)
                emit_gather(c)
            emit_reduce(NCH - 2)
            emit_reduce(NCH - 1)
            return thr_red[NCH - 1]

        thr_gather0 = emit_ag_gather(thr_ag0in, None)

        # h1 = agg + b_gcn ; logits1 = aggWy + (b_gcn@Wy + by) ; zero pad rows
        def _(v):
            e = nc.vector
            e.drain()
            e.tensor_add(out=hv(0, DH), in0=accv(0, DH), in1=bg_rep)
            e.tensor_add(out=hv(DH + C, C), in0=accv(DH + C, C), in1=accv(DH + 2 * C, C))
            e.memset(hv(DH, C), 0.0)
            e.drain()
            e.tensor_add(out=hv(DH + C, C), in0=hv(DH + C, C), in1=c8_rep)
            e.drain()
            dv.inc(e.tensor_tensor(
                out=hv(0, E), in0=hv(0, E),
                in1=AP(padmask, 0, [[M, P], [1, M], [0, E]]),
                op=OP.mult,
            ))
        block.vector(_)
        thr_hup = dv.n

        # ================= GATED ITERATIONS =================
        for it in range(NITER):
            def _(v):
                e = nc.vector
                e.drain()
                e.tensor_reduce(out=sb_mx[:, :], in_=hv(DH + C, C), axis=AX.X, op=OP.max)
                e.drain()
                e.tensor_tensor(
                    out=hv(DH, C), in0=hv(DH + C, C),
                    in1=AP(sb_mx, 0, [[M, P], [1, M], [0, C]]),
                    op=OP.is_equal,
                )
                e.drain()
                e.tensor_tensor(
                    out=hv(DH, C), in0=hv(DH, C),
                    in1=AP(padmask, 0, [[M, P], [1, M], [0, C]]),
                    op=OP.mult,
                )
                e.drain()
                # cast h|onehot|logits_hi to bf16 (cols 0:144)
                e.tensor_copy(
                    out=AP(sb_hb, 0, [[M * EP, P], [EP, M], [1, E]]),
                    in_=hv(0, E),
                )
                e.drain()
                # logits_lo = logits - bf16(logits)
                dv.inc(e.tensor_tensor(
                    out=AP(sb_hb, DH + 2 * C, [[M * EP, P], [EP, M], [1, C]]),
                    in0=hv(DH + C, C),
                    in1=AP(sb_hb, DH + C, [[M * EP, P], [EP, M], [1, C]]),
                    op=OP.subtract,
                ))
            block.vector(_)
            thr_oh = dv.n

            def _(s):
                e = nc.sync
                e.wait_ge(dv.h, thr_oh)
                dS.inc(e.dma_start(
                    out=AP(ag_in, 0, [[EP, P], [P * EP, M], [1, EP]]),
                    in_=sb_hb[:, :, :],
                ))
            block.sync(_)
            thr_agin = dS.n

            thr_gather = emit_ag_gather(thr_agin, thr_hup)

            # ---- f1, f2, local stats ----
            def _(v):
                e = nc.vector
                e.drain()
                e.tensor_tensor(out=sb_t8[:, :, :], in0=accv(DH, C), in1=hv(DH, C), op=OP.mult)
                e.drain()
                e.tensor_reduce(out=sb_f1[:, :], in_=sb_t8[:, :, :], axis=AX.X, op=OP.add)
                e.drain()
                e.tensor_tensor(out=sb_f1[:, :], in0=sb_f1[:, :], in1=padmask[:, :], op=OP.mult)
                dv.inc(e.tensor_scalar_max(out=sb_q[:, :, :], in0=accv(DH, C), scalar1=CLAMP))
            block.vector(_)
            thr_q = dv.n

            def _(a):
                e = nc.scalar
                e.wait_ge(dv.h, thr_q)
                ac.inc(e.activation(out=sb_lnq[:, :, :], in_=sb_q[:, :, :], func=AF.Ln))
            block.scalar(_)
            thr_ln = ac.n

            def _(v):
                e = nc.vector
                e.wait_ge(ac.h, thr_ln)
                e.drain()
                e.tensor_tensor(out=sb_t8[:, :, :], in0=sb_q[:, :, :], in1=sb_lnq[:, :, :], op=OP.mult)
                e.drain()
                e.tensor_reduce(out=sb_f2[:, :], in_=sb_t8[:, :, :], axis=AX.X, op=OP.add, negate=True)
                e.drain()
                dv.inc(e.tensor_tensor(out=sb_f2[:, :], in0=sb_f2[:, :], in1=padmask[:, :], op=OP.mult))
            block.vector(_)
            thr_f2 = dv.n

            def _(a):
                e = nc.scalar
                e.wait_ge(dv.h, thr_f2)
                e.activation(out=sb_junk[:, :], in_=sb_f1[:, :], func=AF.Identity,
                             accum_out=sb_p4[:, 0:1])
                e.drain()
                e.activation(out=sb_junk[:, :], in_=sb_f1[:, :], func=AF.Square,
                             accum_out=sb_p4[:, 1:2])
                e.drain()
                e.activation(out=sb_junk[:, :], in_=sb_f2[:, :], func=AF.Identity,
                             accum_out=sb_p4[:, 2:3])
                e.drain()
                ac.inc(e.activation(out=sb_junk[:, :], in_=sb_f2[:, :], func=AF.Square,
                                    accum_out=sb_p4[:, 3:4]))
            block.scalar(_)
            thr_p4 = ac.n

            def _(t):
                e = nc.tensor
                e.wait_ge(ac.h, thr_p4)
                pe.inc(e.matmul(out=ps_s[0:1, 0:4], lhsT=ones_col[:, :], rhs=sb_p4[:, :],
                                start=True, stop=True))
            block.tensor(_)
            thr_ps = pe.n

            def _(v):
                e = nc.vector
                e.wait_ge(pe.h, thr_ps)
                e.drain()
                dv.inc(e.tensor_copy(out=sb_s4[:, :], in_=ps_s[0:1, 0:4]))
            block.vector(_)
            thr_s4 = dv.n

            def _(s):
                e = nc.sync
                e.wait_ge(dv.h, thr_s4)
                dS.inc(e.dma_start(out=AP(stats_in, 0, [[4, 1], [1, 4]]), in_=sb_s4[:, :]))
            block.sync(_)
            thr_si = dS.n

            def _(g):
                e = nc.gpsimd
                e.wait_ge(dS.h, thr_si)
                cc.inc(e.collective_compute(
                    "AllGather", OP.bypass, replica_groups=rg,
                    ins=[stats_in.ap().opt()], outs=[stats_out.ap().opt()],
                ))
            block.gpsimd(_)
            thr_ccs = cc.n

            def _(s):
                e = nc.sync
                e.wait_ge(cc.h, thr_ccs)
                dS.inc(e.dma_start(out=sb_s32[:, :], in_=AP(stats_out, 0, [[32, 1], [1, 32]])))
            block.sync(_)
            thr_so = dS.n

            def _(v):
                e = nc.vector
                e.wait_ge(dS.h, thr_so)
                e.drain()
                e.tensor_reduce(out=sb_sum4[:, :], in_=AP(sb_s32, 0, [[32, 1], [1, 4], [4, 8]]),
                                axis=AX.X, op=OP.add)
                e.drain()
                e.tensor_scalar_mul(out=sb_mean[:, :], in0=AP(sb_sum4, 0, [[4, 1], [2, 2]]),
                                    scalar1=1.0 / N)
                e.tensor_scalar_mul(out=sb_msq[:, :], in0=AP(sb_sum4, 1, [[4, 1], [2, 2]]),
                                    scalar1=1.0 / N)
                e.drain()
                e.tensor_tensor(out=sb_var[:, :], in0=sb_mean[:, :], in1=sb_mean[:, :], op=OP.mult)
                e.drain()
                e.tensor_tensor(out=sb_var[:, :], in0=sb_msq[:, :], in1=sb_var[:, :], op=OP.subtract)
                e.drain()
                dv.inc(e.tensor_scalar_add(out=sb_var[:, :], in0=sb_var[:, :], scalar1=EPS))
            block.vector(_)
            thr_var = dv.n

            def _(a):
                e = nc.scalar
                e.wait_ge(dv.h, thr_var)
                ac.inc(e.activation(out=sb_sd[:, :], in_=sb_var[:, :], func=AF.Sqrt))
            block.scalar(_)
            thr_sd = ac.n

            def _(v):
                e = nc.vector
                e.wait_ge(ac.h, thr_sd)
                e.drain()
                e.reciprocal(out=sb_rstd[:, :], in_=sb_sd[:, :])
                e.drain()
                e.tensor_scalar_mul(out=sb_nrstd[:, :], in0=sb_rstd[:, :], scalar1=-1.0)
                e.tensor_tensor(out=sb_mr[:, :], in0=sb_mean[:, :], in1=sb_rstd[:, :], op=OP.mult)
                e.drain()
                e.tensor_tensor(out=sb_bias[:, :], in0=sb_mr[:, :], in1=sb_taurow[:, :], op=OP.add)
                e.drain()
                e.tensor_copy(out=AP(sb_pack4, 0, [[4, 1], [2, 2]]), in_=sb_nrstd[:, :])
                e.drain()
                dv.inc(e.tensor_copy(out=AP(sb_pack4, 1, [[4, 1], [2, 2]]), in_=sb_bias[:, :]))
            block.vector(_)
            thr_pack = dv.n

            def _(t):
                e = nc.tensor
                e.wait_ge(dv.h, thr_pack)
                pe.inc(e.matmul(out=ps_mi[:, 0:4], lhsT=ones_row[:, :], rhs=sb_pack4[:, :],
                                start=True, stop=True))
            block.tensor(_)
            thr_bc = pe.n

            def _(v):
                e = nc.vector
                e.wait_ge(pe.h, thr_bc)
                e.drain()
                dv.inc(e.tensor_copy(out=sb_sc[:, :], in_=ps_mi[:, 0:4]))
            block.vector(_)
            thr_sc = dv.n

            def _(a):
                e = nc.scalar
                e.wait_ge(dv.h, thr_sc)
                e.activation(out=sb_z1[:, :], in_=sb_f1[:, :], func=AF.Sigmoid,
                             scale=sb_sc[:, 0:1], bias=sb_sc[:, 1:2])
                ac.inc(e.activation(out=sb_z2[:, :], in_=sb_f2[:, :], func=AF.Sigmoid,
                                    scale=sb_sc[:, 2:3], bias=sb_sc[:, 3:4]))
            block.scalar(_)
            thr_sig = ac.n

            def _(v):
                e = nc.vector
                e.wait_ge(ac.h, thr_sig)
                e.drain()
                e.tensor_tensor(out=sb_z[:, :], in0=sb_z1[:, :], in1=sb_z2[:, :], op=OP.mult)
                e.drain()
                e.tensor_tensor(out=sb_gate[:, :], in0=sb_oldz[:, :], in1=sb_z[:, :], op=OP.min)
                e.drain()
                e.tensor_copy(out=sb_oldz[:, :], in_=sb_z[:, :])
                e.drain()
                e.tensor_tensor(out=sb_big[:, :, :], in0=accv(0, DH), in1=gatev(DH), op=OP.mult)
                e.tensor_tensor(out=sb_t8[:, :, :], in0=accv(DH + C, C), in1=accv(DH + 2 * C, C), op=OP.add)
                e.drain()
                e.tensor_tensor(out=sb_t8[:, :, :], in0=sb_t8[:, :, :], in1=gatev(C), op=OP.mult)
                e.drain()
                e.tensor_tensor(out=hv(0, DH), in0=hv(0, DH), in1=sb_big[:, :, :], op=OP.add)
                e.drain()
                dv.inc(e.tensor_tensor(out=hv(DH + C, C), in0=hv(DH + C, C), in1=sb_t8[:, :, :],
                                       op=OP.add))
            block.vector(_)
            thr_hup = dv.n

        # ================= CLASSIFIER =================
        thr_wld = {}
        thr_tm = {}
        thr_cmm = {}
        for g in (range(M) if not SKIP_CLS else []):
            a = g & 1
            R = P if g < M - 1 else (SH - P * (M - 1))

            def _(s, g=g, a=a, R=R):
                e = nc.sync
                if g >= 2:
                    e.wait_ge(dv.h, thr_tm[g - 2])
                thr_wld[g] = dW[a].inc(e.dma_start(
                    out=sb_wc[a][0:R, :],
                    in_=AP(wc_s, g * P * DH * C, [[DH * C, R], [1, DH * C]]),
                ))
            block.sync(_)

            def _(v, g=g, a=a, R=R):
                e = nc.vector
                e.wait_ge(dW[a].h, thr_wld[g])
                if g >= 2:
                    e.wait_ge(pe.h, thr_cmm[g - 2])
                if g == 0:
                    e.drain()
                thr_tm[g] = dv.inc(e.tensor_tensor(
                    out=AP(sb_tmp[a], 0, [[DH * C, R], [C, DH], [1, C]]),
                    in0=AP(sb_wc[a], 0, [[DH * C, R], [C, DH], [1, C]]),
                    in1=AP(sb_h, g * E, [[hS, R], [1, DH], [0, C]]),
                    op=OP.mult,
                ))
            block.vector(_)

            def _(t, g=g, a=a, R=R):
                e = nc.tensor
                e.wait_ge(dv.h, thr_tm[g])
                if g:
                    e.drain()
                e.matmul(out=ps_c0[0:1, :], lhsT=ones_col[0:R, :], rhs=sb_tmp[a][0:R, 0:512],
                         start=(g == 0), stop=(g == M - 1))
                thr_cmm[g] = pe.inc(e.matmul(
                    out=ps_c1[0:1, :], lhsT=ones_col[0:R, :], rhs=sb_tmp[a][0:R, 512:1024],
                    start=(g == 0), stop=(g == M - 1)))
            block.tensor(_)

        def _(v):
            e = nc.vector
            if SKIP_CLS:
                dv.inc(e.memset(sb_o[:, :], 1.0))
                return
            e.wait_ge(pe.h, thr_cmm[M - 1])
            e.drain()
            e.tensor_reduce(out=sb_r0[:, :], in_=AP(ps_c0, 0, [[512, 1], [1, C], [C, 64]]),
                            axis=AX.X, op=OP.add)
            e.tensor_reduce(out=sb_r1[:, :], in_=AP(ps_c1, 0, [[512, 1], [1, C], [C, 64]]),
                            axis=AX.X, op=OP.add)
            e.drain()
            e.tensor_tensor(out=sb_o[:, :], in0=sb_r0[:, :], in1=sb_r1[:, :], op=OP.add)
            e.drain()
            dv.inc(e.tensor_tensor(out=sb_o[:, :], in0=sb_o[:, :], in1=sb_bc8row[:, :], op=OP.add))
        block.vector(_)
        thr_out = dv.n

        def _(s):
            e = nc.sync
            e.wait_ge(dv.h, thr_out)
            dS.inc(e.dma_start(out=AP(cls_in, 0, [[C, 1], [1, C]]), in_=sb_o[:, :]))
        block.sync(_)
        thr_ci = dS.n

        def _(g):
            e = nc.gpsimd
            e.wait_ge(dS.h, thr_ci)
            cc.inc(e.collective_compute(
                "AllReduce", OP.add, replica_groups=rg,
                ins=[cls_in.ap().opt()], outs=[cls_out.ap().opt()],
            ))
        block.gpsimd(_)
        thr_ccf = cc.n

        def _(s):
            e = nc.sync
            e.wait_ge(cc.h, thr_ccf)
            dS.inc(e.dma_start(out=out_ext[:, :], in_=AP(cls_out, 0, [[C, 1], [1, C]])))
        block.sync(_)

    nc.compile()
    return nc


# ---------------------------------------------------------------------------
# Host side
# ---------------------------------------------------------------------------
_NC_CACHE = None


def _get_nc():
    global _NC_CACHE
    if _NC_CACHE is None:
        _NC_CACHE = build_program()
    return _NC_CACHE


def _wrap_idx(slots):
    """slots [784, 128] (f=(m,k), p) -> dma_gather idx layout [128, 6272] int16.

    Within a chunk of ni idxs, gather element j = f_local*128 + p is read from
    idx16[j % 16, j // 16]; j%16 = p%16, j//16 = f_local*8 + p//16.
    """
    cols = []
    f0 = 0
    while f0 < M * K:
        nf = min(CH_M * K, M * K - f0)
        sub = slots[f0:f0 + nf]                      # [nf, 128]
        sub = sub.reshape(nf, 8, 16).transpose(2, 0, 1).reshape(16, nf * 8)
        cols.append(sub)
        f0 += nf
    w = np.concatenate(cols, axis=1)                  # [16, 6272]
    return np.ascontiguousarray(np.tile(w, (8, 1))).astype(np.int16)


def make_in_maps(inputs):
    feat = np.ascontiguousarray(np.asarray(inputs["feat"], dtype=np.float32))
    neighbors = np.asarray(inputs["neighbors"])
    w_gcn = np.ascontiguousarray(np.asarray(inputs["W_gcn"], dtype=np.float32))
    b_gcn = np.ascontiguousarray(np.asarray(inputs["b_gcn"], dtype=np.float32))
    wy = np.ascontiguousarray(np.asarray(inputs["Wy"], dtype=np.float32))
    by = np.ascontiguousarray(np.asarray(inputs["by"], dtype=np.float32))
    tau1 = np.asarray(inputs["tau1"], dtype=np.float32)
    tau2 = np.asarray(inputs["tau2"], dtype=np.float32)
    wc = np.asarray(inputs["Wc"], dtype=np.float32)
    bc = np.asarray(inputs["bc"], dtype=np.float32)

    taus = np.stack([tau1[0], tau2[0]]).astype(np.float32)
    bc8v = (bc / NCORES).astype(np.float32)

    gmap = (SHP * (neighbors // SH) + (neighbors % SH)).astype(np.int64)  # [N, K]

    in_maps = []
    for r in range(NCORES):
        fs = np.zeros((SHP, DH), np.float32)
        fs[:SH] = feat[r * SH:(r + 1) * SH]
        gg_ = np.zeros((SHP, K), np.int64)
        gg_[:SH] = gmap[r * SH:(r + 1) * SH]
        gg_[SH:] = DUM_A  # pad nodes gather only zeros
        sl = gg_.reshape(M, P, K).transpose(0, 2, 1).reshape(M * K, P)
        slA = np.where(sl < 32768, sl, DUM_A)
        slB = np.where(sl >= 32768, sl - WB, DUM_B)
        wcs = np.ascontiguousarray(wc[r * SH * DH:(r + 1) * SH * DH])
        in_maps.append({
            "feat_s": fs,
            "nbidxA": _wrap_idx(slA),
            "nbidxB": _wrap_idx(slB),
            "w_gcn": w_gcn,
            "b_gcn": b_gcn,
            "wy": wy,
            "by": by,
            "taus": taus,
            "wc_s": wcs,
            "bc8": bc8v,
        })
    return in_maps


def run(inputs, trace=False):
    nc = _get_nc()
    in_maps = make_in_maps(inputs)
    try:
        res = run_bass_kernel_spmd(nc, in_maps, core_ids=list(range(NCORES)), trace=trace)
    except Exception:
        # The axon terminal occasionally drops an execution (worker hang-up);
        # one retry on a fresh invocation has been sufficient in practice.
        import time as _t
        _t.sleep(5.0)
        res = run_bass_kernel_spmd(nc, in_maps, core_ids=list(range(NCORES)), trace=trace)
    out = np.asarray(res.results[0]["out"], dtype=np.float32).reshape(1, C)
    return out, res


def kernel(**inputs) -> np.ndarray:
    out, _ = run(inputs, trace=False)
    return out

